# revision 26
# baseline (speedup 1.0000x reference)
"""Trainium2 Bass kernel for nn_ALRDLinearINT8 (low-rank linear with dynamic
int8 activation quantization), distributed over 8 NeuronCores.

Math (per reference):
    latent = x @ B_w^T                          [B*S, R]
    q, lat_scale = int8_quantize(latent)        per-token symmetric
    aq, a_scale  = int8_quantize(A_w)           per-out-row symmetric
    out = (q @ aq^T) * lat_scale * a_scale^T + A_bias

Strategy: pure data parallelism over the 8192 tokens (1024 tokens/core),
B_w / A_w / A_bias replicated; no collectives.

Host-side marshalling (numerically identical to doing it on device):
  x and B_w are sharded/replicated and handed to each core pre-transposed
  in fp16 (the compute dtype of GEMM1; fp16 cast is the same RNE cast the
  DMA engines would apply). All actual math — both GEMMs, both int8
  quantizations (fp32 amax / scales / round-to-nearest-even), dequant and
  bias — runs on device.

Device notes:
  - GEMM1 in fp16 (full TensorE rate), fp32 PSUM accumulation.
  - Quantization in fp32: amax -> scale, RNE via the 1.5*2^23 magic trick,
    matching jnp.round's round-half-to-even.
  - GEMM2 operands are integers |v| <= 127 stored in fp16, so fp16 matmul
    with fp32 accumulation reproduces the int8 GEMM exactly
    (1024 * 127^2 < 2^24).
  - The quantized-A transpose and the per-token-q transpose use the DMA
    XBAR (2-byte) so the TensorEngine only does GEMM work.
  - GEMM2 computes out^T, which makes a_scale and bias per-partition
    scalars for a fused ScalarE epilogue; lat_scale is broadcast across
    partitions with a tiny ones-vector matmul.
"""

import numpy as np

N_CORES = 8
B_SZ, SEQ = 4, 2048
IN, RANK, OUT = 4096, 1024, 4096
TOK = (B_SZ * SEQ) // N_CORES  # tokens per core = 1024

MAGIC = float(np.float32(1.5 * 2**23))

NT = TOK // 128    # 8 token tiles / core
NI = IN // 128     # 32 contraction tiles for GEMM1
NR = RANK // 128   # 8 contraction tiles for GEMM2
NO = OUT // 128    # 32 output tiles
N_HALF = 2
THALF = TOK // N_HALF          # 512
TT_PER_HALF = THALF // 128     # 4
A_GRP = NO // NT               # A_w o-tiles quantized per token tile = 4

_compiled_nc = None


def _build_nc():
    import concourse.tile as tile
    from concourse import bacc, mybir
    from concourse.bass import ts, ds
    from contextlib import ExitStack

    f32 = mybir.dt.float32
    f16 = mybir.dt.float16
    AX = mybir.AxisListType
    ALU = mybir.AluOpType
    AF = mybir.ActivationFunctionType

    nc = bacc.Bacc("TRN2", target_bir_lowering=False, debug=False)
    x16_d = nc.dram_tensor("x16", [TOK, IN], f16, kind="ExternalInput").ap()
    bwt_d = nc.dram_tensor("B_wT", [IN, RANK], f16, kind="ExternalInput").ap()
    aw_d = nc.dram_tensor("A_w", [OUT, RANK], f32, kind="ExternalInput").ap()
    bias_d = nc.dram_tensor("A_bias", [OUT], f32, kind="ExternalInput").ap()
    ones_d = nc.dram_tensor("ones_row", [1, 128], f32, kind="ExternalInput").ap()
    out_d = nc.dram_tensor("out", [OUT, TOK], f32, kind="ExternalOutput").ap()
    latsd = nc.dram_tensor("latsd", [TOK], f32).ap()  # internal scratch

    with tile.TileContext(nc) as tc, ExitStack() as ctx:
        # ---------- persistent pools (whole kernel) ----------
        constp = ctx.enter_context(tc.tile_pool(name="const", bufs=1))
        wres_a = ctx.enter_context(tc.tile_pool(name="wres_a", bufs=1))
        qtp = ctx.enter_context(tc.tile_pool(name="qtp", bufs=1))
        lsrp = ctx.enter_context(tc.tile_pool(name="lsrp", bufs=1))

        # ---- constants ----
        ones_row = constp.tile([1, 128], f32)
        nc.scalar.dma_start(out=ones_row[:], in_=ones_d)
        # bias in per-partition layout: bias_pp[p, k] = A_bias[k*128 + p]
        bias_pp = constp.tile([128, NO], f32)
        ascale_pp = constp.tile([128, NO], f32)
        magic = constp.tile([128, 1], f32)
        nc.vector.memset(magic[:], MAGIC)

        # aqT[p_r, rt*OUT + o] = aq[o, rt*128 + p_r]
        aqT = wres_a.tile([128, NR * OUT], f16)
        # qT[p_r, rt*TOK + t] = q[t, rt*128 + p_r]
        qT = qtp.tile([128, NR * TOK], f16)

        # ================= phase 1: GEMM1 + quantization =================
        with ExitStack() as p1:
            wres_b = p1.enter_context(tc.tile_pool(name="wres_b", bufs=1))
            xtp = p1.enter_context(tc.tile_pool(name="xtp", bufs=2))
            awp = p1.enter_context(tc.tile_pool(name="awp", bufs=8))
            qa = p1.enter_context(tc.tile_pool(name="qa", bufs=2))
            rnp = p1.enter_context(tc.tile_pool(name="rnp", bufs=4))
            aqp = p1.enter_context(tc.tile_pool(name="aqp", bufs=8))
            smal = p1.enter_context(tc.tile_pool(name="small", bufs=3))
            psA = p1.enter_context(tc.tile_pool(name="psA", bufs=1, space="PSUM"))

            # ---- x transposes: software-pipelined one tile ahead ----
            xTs = {}

            def emit_x_load(tt):
                xT = xtp.tile([128, NI * 128], f16, tag="xT")
                nc.sync.dma_start_transpose(
                    xT[:].rearrange("p (j t) -> p j t", t=128),
                    x16_d[ts(tt, 128), :],
                )
                xTs[tt] = xT

            emit_x_load(0)

            # ---- resident transposed B weights, split for early start ----
            bwTs = []
            for g in range(4):
                bwTg = wres_b.tile([128, (NI // 4) * RANK], f16, name=f"bwT{g}")
                nc.sync.dma_start(
                    out=bwTg[:].rearrange("p (j r) -> p j r", r=RANK),
                    in_=bwt_d.rearrange("(j p) r -> p j r", p=128)[
                        :, ds(g * (NI // 4), NI // 4), :
                    ],
                )
                bwTs.append(bwTg)

            # ---- A_w pipeline pieces ----
            awts = {}

            def emit_a_loads(g):
                for k in range(A_GRP):
                    ot = g * A_GRP + k
                    for h in range(2):
                        awt = awp.tile([128, RANK // 2], f32, tag="awt")
                        nc.gpsimd.dma_start(
                            out=awt[:], in_=aw_d[ts(ot, 128), ts(h, RANK // 2)]
                        )
                        awts[(ot, h)] = awt

            sinv4s = {}

            def emit_a_reduces(g):
                am8 = smal.tile([128, 2 * A_GRP], f32, tag="a_am8")
                for k in range(A_GRP):
                    for h in range(2):
                        nc.vector.tensor_reduce(
                            out=am8[:, 2 * k + h : 2 * k + h + 1],
                            in_=awts[(g * A_GRP + k, h)][:],
                            axis=AX.X, op=ALU.max, apply_absolute_value=True,
                        )
                amax4 = smal.tile([128, A_GRP], f32, tag="a_amax")
                nc.vector.tensor_reduce(
                    out=amax4[:], in_=am8[:].rearrange("p (k h) -> p k h", h=2),
                    axis=AX.X, op=ALU.max,
                )
                amc4 = smal.tile([128, A_GRP], f32, tag="a_amc")
                nc.vector.tensor_scalar_max(amc4[:], amax4[:], 1e-8)
                rec4 = smal.tile([128, A_GRP], f32, tag="a_rec")
                nc.vector.reciprocal(rec4[:], amc4[:])
                sinv4 = smal.tile([128, A_GRP], f32, tag="a_sinv")
                nc.vector.tensor_scalar_mul(sinv4[:], rec4[:], 127.0)
                nc.vector.tensor_scalar_mul(
                    ascale_pp[:, ts(g, A_GRP)], amc4[:], 1.0 / 127.0
                )
                sinv4s[g] = sinv4

            aq16s = {}

            def emit_a_quant_chunks(g):
                sinv4 = sinv4s.pop(g)
                for k in range(A_GRP):
                    ot = g * A_GRP + k
                    aq16 = aqp.tile([128, RANK], f16, tag="aq16")
                    for c in range(RANK // 512):
                        aqt = rnp.tile([128, 512], f32, tag="rndtmp")
                        nc.scalar.activation(
                            out=aqt[:], in_=awts.pop((ot, c))[:], func=AF.Identity,
                            bias=magic[:], scale=sinv4[:, k : k + 1],
                        )
                        nc.vector.tensor_scalar_sub(aq16[:, ts(c, 512)], aqt[:], MAGIC)
                    aq16s[ot] = aq16

            def emit_a_transposes(g):
                for k in range(A_GRP):
                    ot = g * A_GRP + k
                    dst = aqT[:].rearrange("p (j o) -> p j o", o=OUT)[
                        :, :, ts(ot, 128)
                    ]
                    nc.sync.dma_start_transpose(dst, aq16s.pop(ot)[:])

            emit_a_loads(0)
            q16s = {}
            for tt in range(NT):
                if tt + 1 < NT:
                    emit_x_load(tt + 1)
                    emit_a_loads(tt + 1)
                # A reduces early on DVE (their inputs stream in independently)
                emit_a_reduces(tt)
                xT = xTs.pop(tt)
                # GEMM1: latent[t, r] for this 128-token tile
                lat_ps = psA.tile([128, RANK], f32, tag="lat", bufs=3)
                for it in range(NI):
                    lw = xT[:, ts(it, 128)]
                    bg = bwTs[it // (NI // 4)]
                    ib = it % (NI // 4)
                    for rc in range(RANK // 512):
                        nc.tensor.matmul(
                            lat_ps[:, ts(rc, 512)],
                            lw,
                            bg[:, ib * RANK + rc * 512 : ib * RANK + (rc + 1) * 512],
                            start=(it == 0),
                            stop=(it == NI - 1),
                        )
                # per-token quantization (fp32, RNE via magic)
                amax = smal.tile([128, 1], f32, tag="amax")
                nc.vector.tensor_reduce(
                    out=amax[:], in_=lat_ps[:], axis=AX.X, op=ALU.max,
                    apply_absolute_value=True,
                )
                amc = smal.tile([128, 1], f32, tag="amc")
                nc.vector.tensor_scalar_max(amc[:], amax[:], 1e-8)
                rec = smal.tile([128, 1], f32, tag="rec")
                nc.vector.reciprocal(rec[:], amc[:])
                sinv = smal.tile([128, 1], f32, tag="sinv")
                nc.vector.tensor_scalar_mul(sinv[:], rec[:], 127.0)
                # amc -> DRAM; phase 2 rebuilds lat_scale rows from it
                nc.gpsimd.dma_start(out=latsd[ts(tt, 128)], in_=amc[:, 0:1])
                q16 = qa.tile([128, RANK], f16, tag="q16")
                for c in range(RANK // 512):
                    qt32 = rnp.tile([128, 512], f32, tag="rndtmp")
                    nc.scalar.activation(
                        out=qt32[:], in_=lat_ps[:, ts(c, 512)], func=AF.Identity,
                        bias=magic[:], scale=sinv[:],
                    )
                    nc.vector.tensor_scalar_sub(q16[:, ts(c, 512)], qt32[:], MAGIC)
                q16s[tt] = q16
                # A-path tail work, lagged where it feeds the sync queue
                emit_a_quant_chunks(tt)
                if tt > 0:
                    emit_a_transposes(tt - 1)
                    nc.sync.dma_start_transpose(
                        qT[:].rearrange("p (j t) -> p j t", t=TOK)[
                            :, :, ts(tt - 1, 128)
                        ],
                        q16s.pop(tt - 1)[:],
                    )
            nc.sync.dma_start_transpose(
                qT[:].rearrange("p (j t) -> p j t", t=TOK)[:, :, ts(NT - 1, 128)],
                q16s.pop(NT - 1)[:],
            )
            emit_a_transposes(NT - 1)

        # ================= phase 2: GEMM2 (out^T) + dequant =================
        with ExitStack() as p2:
            outp = p2.enter_context(tc.tile_pool(name="outp", bufs=4))
            lsp = p2.enter_context(tc.tile_pool(name="lsp", bufs=1))
            psB = p2.enter_context(tc.tile_pool(name="psB", bufs=1, space="PSUM"))
            # broadcast lat_scale over all partitions: lsb[p, t] = amc[t]/127
            lsrow = lsp.tile([1, TOK], f32, tag="lsrow")
            nc.sync.dma_start(out=lsrow[:], in_=latsd[None, :])
            bc_ps = psB.tile([128, TOK], f32, tag="bcps", bufs=1)
            for h in range(TOK // 512):
                nc.tensor.matmul(
                    bc_ps[:, ts(h, 512)], ones_row[:], lsrow[0:1, ts(h, 512)],
                    start=True, stop=True,
                )
            lsb = lsp.tile([128, TOK], f32, tag="lsb")
            nc.scalar.activation(
                out=lsb[:], in_=bc_ps[:], func=AF.Copy, scale=1.0 / 127.0
            )
            nc.scalar.dma_start(
                out=bias_pp[:], in_=bias_d.rearrange("(k p) -> p k", p=128)
            )
            for ot in range(NO):
                ops = psB.tile([128, TOK], f32, tag="out", bufs=3)
                for rt in range(NR):
                    lw = aqT[:, rt * OUT + ot * 128 : rt * OUT + (ot + 1) * 128]
                    for h in range(TOK // 512):
                        nc.tensor.matmul(
                            ops[:, ts(h, 512)],
                            lw,
                            qT[:, rt * TOK + h * 512 : rt * TOK + (h + 1) * 512],
                            start=(rt == 0),
                            stop=(rt == NR - 1),
                        )
                tmp = outp.tile([128, TOK], f32, tag="deq")
                nc.vector.tensor_tensor(tmp[:], ops[:], lsb[:], ALU.mult)
                ob = outp.tile([128, TOK], f32, tag="deq")
                nc.scalar.activation(
                    out=ob[:], in_=tmp[:], func=AF.Identity,
                    bias=bias_pp[:, ot : ot + 1], scale=ascale_pp[:, ot : ot + 1],
                )
                nc.sync.dma_start(out=out_d[ts(ot, 128), :], in_=ob[:])

    nc.compile()
    return nc


def _get_nc():
    global _compiled_nc
    if _compiled_nc is None:
        _compiled_nc = _build_nc()
    return _compiled_nc


def _make_in_maps(x, B_w, A_w, A_bias):
    x = np.asarray(x, dtype=np.float32).reshape(-1, IN)
    B_w = np.asarray(B_w, dtype=np.float32)
    A_w = np.ascontiguousarray(np.asarray(A_w, dtype=np.float32))
    A_bias = np.ascontiguousarray(np.asarray(A_bias, dtype=np.float32))
    bwt16 = np.ascontiguousarray(B_w.astype(np.float16).T)  # [IN, RANK]
    ones_row = np.ones((1, 128), dtype=np.float32)
    in_maps = []
    for c in range(N_CORES):
        x16 = np.ascontiguousarray(x[c * TOK : (c + 1) * TOK].astype(np.float16))
        in_maps.append(
            {
                "x16": x16,
                "B_wT": bwt16,
                "A_w": A_w,
                "A_bias": A_bias,
                "ones_row": ones_row,
            }
        )
    return in_maps


def _run(inputs, trace=False, trace_kwargs=None):
    from concourse.bass_utils import run_bass_kernel_spmd

    nc = _get_nc()
    in_maps = _make_in_maps(
        inputs["x"], inputs["B_w"], inputs["A_w"], inputs["A_bias"]
    )
    res = run_bass_kernel_spmd(
        nc, in_maps, core_ids=list(range(N_CORES)), trace=trace,
        **(trace_kwargs or {}),
    )
    parts = [res.results[c]["out"].T for c in range(N_CORES)]  # each [TOK, OUT]
    out = np.concatenate(parts, axis=0).reshape(B_SZ, SEQ, OUT)
    return np.ascontiguousarray(out.astype(np.float32)), res


def kernel(**inputs) -> np.ndarray:
    out, _ = _run(inputs, trace=False)
    return out


# revision 27
# speedup vs baseline: 1.5480x; 1.5480x over previous
"""Trainium2 Bass kernel for nn_ALRDLinearINT8 (low-rank linear with dynamic
int8 activation quantization), distributed over 8 NeuronCores.

Math (per reference):
    latent = x @ B_w^T                          [B*S, R]
    q, lat_scale = int8_quantize(latent)        per-token symmetric
    aq, a_scale  = int8_quantize(A_w)           per-out-row symmetric
    out = (q @ aq^T) * lat_scale * a_scale^T + A_bias

Strategy: pure data parallelism over the 8192 tokens (1024 tokens/core),
B_w / A_w / A_bias replicated; no collectives.

Host-side marshalling (numerically identical to doing it on device):
  x and B_w are sharded/replicated and handed to each core pre-transposed
  in fp16 (the compute dtype of GEMM1; the fp16 cast is the same RNE cast
  the DMA engines would apply). All actual math — both GEMMs, both int8
  quantizations (fp32 amax / scales / round-to-nearest-even), dequant and
  bias — runs on device.

Device notes:
  - GEMM1 in fp16 (full TensorE rate), fp32 PSUM accumulation.
  - Quantization in fp32: amax -> scale, RNE via the 1.5*2^23 magic trick,
    matching jnp.round's round-half-to-even.
  - GEMM2 operands are integers |v| <= 127 stored in fp16, so fp16 matmul
    with fp32 accumulation reproduces the int8 GEMM exactly
    (1024 * 127^2 < 2^24).
  - The quantized-A transpose and the per-token-q transpose use the DMA
    XBAR (2-byte) so the TensorEngine only does GEMM work.
  - GEMM2 computes out^T, which makes a_scale and bias per-partition
    scalars for a fused ScalarE epilogue; lat_scale is broadcast across
    partitions with a tiny ones-vector matmul.
"""

import numpy as np

N_CORES = 8
B_SZ, SEQ = 4, 2048
IN, RANK, OUT = 4096, 1024, 4096
TOK = (B_SZ * SEQ) // N_CORES  # tokens per core = 1024

MAGIC = float(np.float32(1.5 * 2**23))

NT = TOK // 128    # 8 token tiles / core
NI = IN // 128     # 32 contraction tiles for GEMM1
NR = RANK // 128   # 8 contraction tiles for GEMM2
NO = OUT // 128    # 32 output tiles
N_HALF = 2
THALF = TOK // N_HALF          # 512
TT_PER_HALF = THALF // 128     # 4
A_GRP = NO // NT               # A_w o-tiles quantized per token tile = 4

_compiled_nc = None


def _build_nc():
    import concourse.tile as tile
    from concourse import bacc, mybir
    from concourse.bass import ts, ds
    from contextlib import ExitStack

    f32 = mybir.dt.float32
    f16 = mybir.dt.float16
    AX = mybir.AxisListType
    ALU = mybir.AluOpType
    AF = mybir.ActivationFunctionType

    nc = bacc.Bacc("TRN2", target_bir_lowering=False, debug=False)
    xt_d = nc.dram_tensor("xT", [IN, TOK], f16, kind="ExternalInput").ap()
    bwt_d = nc.dram_tensor("B_wT", [IN, RANK], f16, kind="ExternalInput").ap()
    aw_d = nc.dram_tensor("A_w", [OUT, RANK], f32, kind="ExternalInput").ap()
    bias_d = nc.dram_tensor("A_bias", [OUT], f32, kind="ExternalInput").ap()
    ident_d = nc.dram_tensor("ident", [128, 128], f32, kind="ExternalInput").ap()
    ones_d = nc.dram_tensor("ones_row", [1, 128], f32, kind="ExternalInput").ap()
    out_d = nc.dram_tensor("out", [OUT, TOK], f32, kind="ExternalOutput").ap()

    with tile.TileContext(nc) as tc, ExitStack() as ctx:
        constp = ctx.enter_context(tc.tile_pool(name="const", bufs=1))
        wres = ctx.enter_context(tc.tile_pool(name="wres", bufs=1))
        xtp = ctx.enter_context(tc.tile_pool(name="xtp", bufs=2))
        qtp = ctx.enter_context(tc.tile_pool(name="qtp", bufs=2))
        awp = ctx.enter_context(tc.tile_pool(name="awp", bufs=2))
        qa = ctx.enter_context(tc.tile_pool(name="qa", bufs=2))
        aqp = ctx.enter_context(tc.tile_pool(name="aqp", bufs=4))
        smal = ctx.enter_context(tc.tile_pool(name="small", bufs=2))
        outp = ctx.enter_context(tc.tile_pool(name="outp", bufs=2))
        lsp = ctx.enter_context(tc.tile_pool(name="lsp", bufs=2))
        lsrp = ctx.enter_context(tc.tile_pool(name="lsrp", bufs=1))
        ps_lat = ctx.enter_context(tc.tile_pool(name="ps_lat", bufs=2, space="PSUM"))
        ps_out = ctx.enter_context(tc.tile_pool(name="ps_out", bufs=2, space="PSUM"))
        ps_misc = ctx.enter_context(tc.tile_pool(name="ps_misc", bufs=1, space="PSUM"))

        # ---- constants ----
        ident = constp.tile([128, 128], f32)
        nc.sync.dma_start(out=ident[:], in_=ident_d)
        ones_row = constp.tile([1, 128], f32)
        nc.sync.dma_start(out=ones_row[:], in_=ones_d)
        # bias in per-partition layout: bias_pp[p, k] = A_bias[k*128 + p]
        bias_pp = constp.tile([128, NO], f32)
        nc.sync.dma_start(out=bias_pp[:], in_=bias_d.rearrange("(k p) -> p k", p=128))
        ascale_pp = constp.tile([128, NO], f32)
        magic = constp.tile([128, 1], f32)
        nc.vector.memset(magic[:], MAGIC)
        negmagic = constp.tile([128, 1], f32)
        nc.vector.memset(negmagic[:], -MAGIC)

        # ---- resident transposed weights (fp16) ----
        # bwT[p_i, it*RANK + r] = B_w[r, it*128 + p_i]
        bwT = wres.tile([128, NI * RANK], f16)
        for g in range(4):
            nc.sync.dma_start(
                out=bwT[:].rearrange("p (j r) -> p j r", r=RANK)[
                    :, ds(g * (NI // 4), NI // 4), :
                ],
                in_=bwt_d.rearrange("(j p) r -> p j r", p=128)[
                    :, ds(g * (NI // 4), NI // 4), :
                ],
            )
        # aqT[p_r, rt*OUT + o] = aq[o, rt*128 + p_r]
        aqT = wres.tile([128, NR * OUT], f16)

        # ---- A_w loads: early, on the (otherwise idle) gpsimd SWDGE queue ----
        awts = []
        for ot in range(NO):
            awt = awp.tile([128, RANK], f32, tag="awt")
            nc.gpsimd.dma_start(out=awt[:], in_=aw_d[ts(ot, 128), :])
            awts.append(awt)

        # ---- A_w quantization, emitted in groups interleaved with GEMM1 ----
        aq16s = {}

        def emit_a_quant_group(g):
            for k in range(A_GRP):
                ot = g * A_GRP + k
                amax = smal.tile([128, 1], f32, tag="a_amax")
                nc.vector.tensor_reduce(
                    out=amax[:], in_=awts[ot][:], axis=AX.X,
                    op=ALU.max, apply_absolute_value=True,
                )
                amc = smal.tile([128, 1], f32, tag="a_amc")
                nc.vector.tensor_scalar_max(amc[:], amax[:], 1e-8)
                rec = smal.tile([128, 1], f32, tag="a_rec")
                nc.vector.reciprocal(rec[:], amc[:])
                sinv = smal.tile([128, 1], f32, tag="a_sinv")
                nc.vector.tensor_scalar_mul(sinv[:], rec[:], 127.0)
                nc.vector.tensor_scalar_mul(
                    ascale_pp[:, ot : ot + 1], amc[:], 1.0 / 127.0
                )
                aq16 = aqp.tile([128, RANK], f16, tag="aq16")
                for c in range(RANK // 512):
                    aqt = qa.tile([128, 512], f32, tag="aqtmp")
                    nc.scalar.activation(
                        out=aqt[:], in_=awts[ot][:, ts(c, 512)], func=AF.Identity,
                        bias=magic[:], scale=sinv[:],
                    )
                    nc.scalar.activation(
                        out=aq16[:, ts(c, 512)], in_=aqt[:], func=AF.Identity,
                        bias=negmagic[:],
                    )
                aq16s[ot] = aq16

        def emit_a_transpose_group(g):
            for k in range(A_GRP):
                ot = g * A_GRP + k
                dst = aqT[:].rearrange("p (j o) -> p j o", o=OUT)[:, :, ts(ot, 128)]
                nc.sync.dma_start_transpose(dst, aq16s.pop(ot)[:])

        # ---- phase 1: GEMM1 + per-token quantization for ALL tiles ----
        qTs = []
        lsrows = []
        for th in range(N_HALF):
            qT = qtp.tile([128, NR * THALF], f16)
            lsrow = lsrp.tile([1, THALF], f32, tag="lsrow")
            for tl in range(TT_PER_HALF):
                tt = th * TT_PER_HALF + tl
                xT = xtp.tile([128, NI * 128], f16)
                nc.sync.dma_start(
                    out=xT[:].rearrange("p (j t) -> p j t", t=128),
                    in_=xt_d.rearrange("(j p) t -> p j t", p=128)[:, :, ts(tt, 128)],
                )
                # GEMM1: latent[t, r] for this 128-token tile
                lat_ps = ps_lat.tile([128, RANK], f32)
                for it in range(NI):
                    lw = xT[:, ts(it, 128)]
                    for rc in range(RANK // 512):
                        nc.tensor.matmul(
                            lat_ps[:, ts(rc, 512)],
                            lw,
                            bwT[:, it * RANK + rc * 512 : it * RANK + (rc + 1) * 512],
                            start=(it == 0),
                            stop=(it == NI - 1),
                        )
                # per-token quantization
                amax = smal.tile([128, 1], f32, tag="amax")
                nc.vector.tensor_reduce(
                    out=amax[:], in_=lat_ps[:], axis=AX.X, op=ALU.max,
                    apply_absolute_value=True,
                )
                amc = smal.tile([128, 1], f32, tag="amc")
                nc.vector.tensor_scalar_max(amc[:], amax[:], 1e-8)
                rec = smal.tile([128, 1], f32, tag="rec")
                nc.vector.reciprocal(rec[:], amc[:])
                sinv = smal.tile([128, 1], f32, tag="sinv")
                nc.vector.tensor_scalar_mul(sinv[:], rec[:], 127.0)
                lat_s = smal.tile([128, 1], f32, tag="lats")
                nc.vector.tensor_scalar_mul(lat_s[:], amc[:], 1.0 / 127.0)
                q16 = qa.tile([128, RANK], f16, tag="q16")
                for c in range(RANK // 512):
                    qt32 = qa.tile([128, 512], f32, tag="qtmp")
                    nc.scalar.activation(
                        out=qt32[:], in_=lat_ps[:, ts(c, 512)], func=AF.Identity,
                        bias=magic[:], scale=sinv[:],
                    )
                    nc.vector.tensor_scalar_sub(q16[:, ts(c, 512)], qt32[:], MAGIC)
                nc.sync.dma_start_transpose(
                    qT[:].rearrange("p (j t) -> p j t", t=THALF)[:, :, ts(tl, 128)],
                    q16[:],
                )
                # lat_s -> row vector (PE transpose via identity matmul)
                ls_ps = ps_misc.tile([1, 128], f32, tag="lsps")
                nc.tensor.matmul(ls_ps[:], lat_s[:], ident[:], start=True, stop=True)
                nc.scalar.copy(lsrow[0:1, ts(tl, 128)], ls_ps[:])
                # interleave A-path: quantize group tt, transpose group tt-1
                emit_a_quant_group(tt)
                if tt > 0:
                    emit_a_transpose_group(tt - 1)
            qTs.append(qT)
            lsrows.append(lsrow)
        emit_a_transpose_group(NT - 1)

        # ---- phase 2: GEMM2 (out^T) + dequant epilogue ----
        for th in range(N_HALF):
            qT = qTs[th]
            bc_ps = ps_misc.tile([128, THALF], f32, tag="bcps")
            nc.tensor.matmul(
                bc_ps[:], ones_row[:], lsrows[th][:], start=True, stop=True
            )
            lsb = lsp.tile([128, THALF], f32, tag="lsb")
            nc.scalar.copy(lsb[:], bc_ps[:])
            for ot in range(NO):
                ops = ps_out.tile([128, THALF], f32)
                for rt in range(NR):
                    nc.tensor.matmul(
                        ops[:],
                        aqT[:, rt * OUT + ot * 128 : rt * OUT + (ot + 1) * 128],
                        qT[:, ts(rt, THALF)],
                        start=(rt == 0),
                        stop=(rt == NR - 1),
                    )
                tmp = outp.tile([128, THALF], f32, tag="deq1")
                nc.vector.tensor_tensor(tmp[:], ops[:], lsb[:], ALU.mult)
                ob = outp.tile([128, THALF], f32, tag="deq2")
                nc.scalar.activation(
                    out=ob[:], in_=tmp[:], func=AF.Identity,
                    bias=bias_pp[:, ot : ot + 1], scale=ascale_pp[:, ot : ot + 1],
                )
                nc.sync.dma_start(
                    out=out_d[ts(ot, 128), ds(th * THALF, THALF)], in_=ob[:]
                )

    nc.compile()
    return nc


def _get_nc():
    global _compiled_nc
    if _compiled_nc is None:
        _compiled_nc = _build_nc()
    return _compiled_nc


def _make_in_maps(x, B_w, A_w, A_bias):
    x = np.asarray(x, dtype=np.float32).reshape(-1, IN)
    B_w = np.asarray(B_w, dtype=np.float32)
    A_w = np.ascontiguousarray(np.asarray(A_w, dtype=np.float32))
    A_bias = np.ascontiguousarray(np.asarray(A_bias, dtype=np.float32))
    bwt16 = np.ascontiguousarray(B_w.astype(np.float16).T)  # [IN, RANK]
    ident = np.eye(128, dtype=np.float32)
    ones_row = np.ones((1, 128), dtype=np.float32)
    in_maps = []
    for c in range(N_CORES):
        xt16 = np.ascontiguousarray(
            x[c * TOK : (c + 1) * TOK].astype(np.float16).T
        )  # [IN, TOK]
        in_maps.append(
            {
                "xT": xt16,
                "B_wT": bwt16,
                "A_w": A_w,
                "A_bias": A_bias,
                "ident": ident,
                "ones_row": ones_row,
            }
        )
    return in_maps


def _run(inputs, trace=False, trace_kwargs=None):
    from concourse.bass_utils import run_bass_kernel_spmd

    nc = _get_nc()
    in_maps = _make_in_maps(
        inputs["x"], inputs["B_w"], inputs["A_w"], inputs["A_bias"]
    )
    res = run_bass_kernel_spmd(
        nc, in_maps, core_ids=list(range(N_CORES)), trace=trace,
        **(trace_kwargs or {}),
    )
    parts = [res.results[c]["out"].T for c in range(N_CORES)]  # each [TOK, OUT]
    out = np.concatenate(parts, axis=0).reshape(B_SZ, SEQ, OUT)
    return np.ascontiguousarray(out.astype(np.float32)), res


def kernel(**inputs) -> np.ndarray:
    out, _ = _run(inputs, trace=False)
    return out


# revision 40
# speedup vs baseline: 1.5503x; 1.0015x over previous
"""Trainium2 Bass kernel for nn_ALRDLinearINT8 (low-rank linear with dynamic
int8 activation quantization), distributed over 8 NeuronCores.

Math (per reference):
    latent = x @ B_w^T                          [B*S, R]
    q, lat_scale = int8_quantize(latent)        per-token symmetric
    aq, a_scale  = int8_quantize(A_w)           per-out-row symmetric
    out = (q @ aq^T) * lat_scale * a_scale^T + A_bias

Strategy: pure data parallelism over the 8192 tokens (1024 tokens/core),
B_w / A_w / A_bias replicated; no collectives.

Host-side marshalling (numerically identical to doing it on device):
  x and B_w are sharded/replicated and handed to each core pre-transposed
  in fp16 (the compute dtype of GEMM1; the fp16 cast is the same RNE cast
  the DMA engines would apply). All actual math — both GEMMs, both int8
  quantizations (fp32 amax / scales / round-to-nearest-even), dequant and
  bias — runs on device.

Device notes:
  - GEMM1 in fp16 (full TensorE rate), fp32 PSUM accumulation.
  - Quantization in fp32: amax -> scale, RNE via the 1.5*2^23 magic trick,
    matching jnp.round's round-half-to-even.
  - GEMM2 operands are integers |v| <= 127 stored in fp16, so fp16 matmul
    with fp32 accumulation reproduces the int8 GEMM exactly
    (1024 * 127^2 < 2^24).
  - The quantized-A transpose and the per-token-q transpose use the DMA
    XBAR (2-byte) so the TensorEngine only does GEMM work.
  - GEMM2 computes out^T, which makes a_scale and bias per-partition
    scalars for a fused ScalarE epilogue; lat_scale is broadcast across
    partitions with a tiny ones-vector matmul.
"""

import numpy as np

N_CORES = 8
B_SZ, SEQ = 4, 2048
IN, RANK, OUT = 4096, 1024, 4096
TOK = (B_SZ * SEQ) // N_CORES  # tokens per core = 1024

MAGIC = float(np.float32(1.5 * 2**23))

NT = TOK // 128    # 8 token tiles / core
NI = IN // 128     # 32 contraction tiles for GEMM1
NR = RANK // 128   # 8 contraction tiles for GEMM2
NO = OUT // 128    # 32 output tiles
N_HALF = 2
THALF = TOK // N_HALF          # 512
TT_PER_HALF = THALF // 128     # 4
A_GRP = NO // NT               # A_w o-tiles quantized per token tile = 4

_compiled_nc = None


def _build_nc():
    import concourse.tile as tile
    from concourse import bacc, mybir
    from concourse.bass import ts, ds
    from contextlib import ExitStack

    f32 = mybir.dt.float32
    f16 = mybir.dt.float16
    AX = mybir.AxisListType
    ALU = mybir.AluOpType
    AF = mybir.ActivationFunctionType

    nc = bacc.Bacc("TRN2", target_bir_lowering=False, debug=False)
    xt_d = nc.dram_tensor("xT", [IN, TOK], f16, kind="ExternalInput").ap()
    bwt_d = nc.dram_tensor("B_wT", [IN, RANK], f16, kind="ExternalInput").ap()
    aw_d = nc.dram_tensor("A_w", [OUT, RANK], f32, kind="ExternalInput").ap()
    bias_d = nc.dram_tensor("A_bias", [OUT], f32, kind="ExternalInput").ap()
    ident_d = nc.dram_tensor("ident", [128, 128], f32, kind="ExternalInput").ap()
    ones_d = nc.dram_tensor("ones_row", [1, 128], f32, kind="ExternalInput").ap()
    out_d = nc.dram_tensor("out", [OUT, TOK], f32, kind="ExternalOutput").ap()

    with tile.TileContext(nc) as tc, ExitStack() as ctx:
        constp = ctx.enter_context(tc.tile_pool(name="const", bufs=1))
        wres = ctx.enter_context(tc.tile_pool(name="wres", bufs=1))
        xtp = ctx.enter_context(tc.tile_pool(name="xtp", bufs=2))
        qtp = ctx.enter_context(tc.tile_pool(name="qtp", bufs=2))
        awp = ctx.enter_context(tc.tile_pool(name="awp", bufs=2))
        qa = ctx.enter_context(tc.tile_pool(name="qa", bufs=2))
        aqp = ctx.enter_context(tc.tile_pool(name="aqp", bufs=4))
        smal = ctx.enter_context(tc.tile_pool(name="small", bufs=2))
        outp = ctx.enter_context(tc.tile_pool(name="outp", bufs=2))
        lsp = ctx.enter_context(tc.tile_pool(name="lsp", bufs=2))
        lsrp = ctx.enter_context(tc.tile_pool(name="lsrp", bufs=1))
        ps_lat = ctx.enter_context(tc.tile_pool(name="ps_lat", bufs=2, space="PSUM"))
        ps_out = ctx.enter_context(tc.tile_pool(name="ps_out", bufs=2, space="PSUM"))
        ps_misc = ctx.enter_context(tc.tile_pool(name="ps_misc", bufs=1, space="PSUM"))

        # ---- constants ----
        ident = constp.tile([128, 128], f32)
        nc.sync.dma_start(out=ident[:], in_=ident_d)
        ones_row = constp.tile([1, 128], f32)
        nc.sync.dma_start(out=ones_row[:], in_=ones_d)
        # bias in per-partition layout: bias_pp[p, k] = A_bias[k*128 + p]
        bias_pp = constp.tile([128, NO], f32)
        nc.sync.dma_start(out=bias_pp[:], in_=bias_d.rearrange("(k p) -> p k", p=128))
        ascale_pp = constp.tile([128, NO], f32)
        magic = constp.tile([128, 1], f32)
        nc.vector.memset(magic[:], MAGIC)
        negmagic = constp.tile([128, 1], f32)
        nc.vector.memset(negmagic[:], -MAGIC)

        # ---- resident transposed weights (fp16) ----
        # bwT[p_i, it*RANK + r] = B_w[r, it*128 + p_i]
        bwT = wres.tile([128, NI * RANK], f16)
        for g in range(4):
            nc.sync.dma_start(
                out=bwT[:].rearrange("p (j r) -> p j r", r=RANK)[
                    :, ds(g * (NI // 4), NI // 4), :
                ],
                in_=bwt_d.rearrange("(j p) r -> p j r", p=128)[
                    :, ds(g * (NI // 4), NI // 4), :
                ],
            )
        # aqT[p_r, rt*OUT + o] = aq[o, rt*128 + p_r]
        aqT = wres.tile([128, NR * OUT], f16)

        # ---- A_w loads: early, on the (otherwise idle) gpsimd SWDGE queue ----
        awts = []
        for ot in range(NO):
            awt = awp.tile([128, RANK], f32, tag="awt")
            nc.gpsimd.dma_start(out=awt[:], in_=aw_d[ts(ot, 128), :])
            awts.append(awt)

        # ---- A_w quantization, emitted in groups interleaved with GEMM1 ----
        aq16s = {}

        def emit_a_quant_group(g):
            for k in range(A_GRP):
                ot = g * A_GRP + k
                amax = smal.tile([128, 1], f32, tag="a_amax")
                nc.vector.tensor_reduce(
                    out=amax[:], in_=awts[ot][:], axis=AX.X,
                    op=ALU.max, apply_absolute_value=True,
                )
                amc = smal.tile([128, 1], f32, tag="a_amc")
                nc.vector.tensor_scalar_max(amc[:], amax[:], 1e-8)
                rec = smal.tile([128, 1], f32, tag="a_rec")
                nc.vector.reciprocal(rec[:], amc[:])
                sinv = smal.tile([128, 1], f32, tag="a_sinv")
                nc.vector.tensor_scalar_mul(sinv[:], rec[:], 127.0)
                nc.vector.tensor_scalar_mul(
                    ascale_pp[:, ot : ot + 1], amc[:], 1.0 / 127.0
                )
                aq16 = aqp.tile([128, RANK], f16, tag="aq16")
                for c in range(RANK // 512):
                    aqt = qa.tile([128, 512], f32, tag="aqtmp")
                    nc.scalar.activation(
                        out=aqt[:], in_=awts[ot][:, ts(c, 512)], func=AF.Identity,
                        bias=magic[:], scale=sinv[:],
                    )
                    nc.scalar.activation(
                        out=aq16[:, ts(c, 512)], in_=aqt[:], func=AF.Identity,
                        bias=negmagic[:],
                    )
                aq16s[ot] = aq16

        def emit_a_transpose_group(g):
            for k in range(A_GRP):
                ot = g * A_GRP + k
                dst = aqT[:].rearrange("p (j o) -> p j o", o=OUT)[:, :, ts(ot, 128)]
                nc.sync.dma_start_transpose(dst, aq16s.pop(ot)[:])

        # ---- phase 1: GEMM1 + per-token quantization for ALL tiles ----
        xTs = {}

        def emit_x_load(tt):
            xT = xtp.tile([128, NI * 128], f16, tag="xT")
            nc.sync.dma_start(
                out=xT[:].rearrange("p (j t) -> p j t", t=128),
                in_=xt_d.rearrange("(j p) t -> p j t", p=128)[:, :, ts(tt, 128)],
            )
            xTs[tt] = xT

        emit_x_load(0)
        qTs = []
        lsrows = []
        for th in range(N_HALF):
            qT = qtp.tile([128, NR * THALF], f16)
            lsrow = lsrp.tile([1, THALF], f32, tag="lsrow")
            for tl in range(TT_PER_HALF):
                tt = th * TT_PER_HALF + tl
                if tt + 1 < NT:
                    emit_x_load(tt + 1)
                xT = xTs.pop(tt)
                # GEMM1: latent[t, r] for this 128-token tile
                lat_ps = ps_lat.tile([128, RANK], f32)
                for it in range(NI):
                    lw = xT[:, ts(it, 128)]
                    for rc in range(RANK // 512):
                        nc.tensor.matmul(
                            lat_ps[:, ts(rc, 512)],
                            lw,
                            bwT[:, it * RANK + rc * 512 : it * RANK + (rc + 1) * 512],
                            start=(it == 0),
                            stop=(it == NI - 1),
                        )
                # per-token quantization
                amax = smal.tile([128, 1], f32, tag="amax")
                nc.vector.tensor_reduce(
                    out=amax[:], in_=lat_ps[:], axis=AX.X, op=ALU.max,
                    apply_absolute_value=True,
                )
                amc = smal.tile([128, 1], f32, tag="amc")
                nc.vector.tensor_scalar_max(amc[:], amax[:], 1e-8)
                rec = smal.tile([128, 1], f32, tag="rec")
                nc.vector.reciprocal(rec[:], amc[:])
                sinv = smal.tile([128, 1], f32, tag="sinv")
                nc.vector.tensor_scalar_mul(sinv[:], rec[:], 127.0)
                lat_s = smal.tile([128, 1], f32, tag="lats")
                nc.vector.tensor_scalar_mul(lat_s[:], amc[:], 1.0 / 127.0)
                q16 = qa.tile([128, RANK], f16, tag="q16")
                for c in range(RANK // 512):
                    qt32 = qa.tile([128, 512], f32, tag="qtmp")
                    nc.scalar.activation(
                        out=qt32[:], in_=lat_ps[:, ts(c, 512)], func=AF.Identity,
                        bias=magic[:], scale=sinv[:],
                    )
                    nc.vector.tensor_scalar_sub(q16[:, ts(c, 512)], qt32[:], MAGIC)
                nc.sync.dma_start_transpose(
                    qT[:].rearrange("p (j t) -> p j t", t=THALF)[:, :, ts(tl, 128)],
                    q16[:],
                )
                # lat_s -> row vector (PE transpose via identity matmul)
                ls_ps = ps_misc.tile([1, 128], f32, tag="lsps")
                nc.tensor.matmul(ls_ps[:], lat_s[:], ident[:], start=True, stop=True)
                nc.scalar.copy(lsrow[0:1, ts(tl, 128)], ls_ps[:])
                # interleave A-path: quantize group tt, transpose group tt-1
                emit_a_quant_group(tt)
                if tt > 0:
                    emit_a_transpose_group(tt - 1)
            qTs.append(qT)
            lsrows.append(lsrow)
        emit_a_transpose_group(NT - 1)

        # ---- phase 2: GEMM2 (out^T) + dequant epilogue ----
        for th in range(N_HALF):
            qT = qTs[th]
            bc_ps = ps_misc.tile([128, THALF], f32, tag="bcps")
            nc.tensor.matmul(
                bc_ps[:], ones_row[:], lsrows[th][:], start=True, stop=True
            )
            lsb = lsp.tile([128, THALF], f32, tag="lsb")
            nc.scalar.copy(lsb[:], bc_ps[:])
            for ot in range(NO):
                ops = ps_out.tile([128, THALF], f32)
                for rt in range(NR):
                    nc.tensor.matmul(
                        ops[:],
                        aqT[:, rt * OUT + ot * 128 : rt * OUT + (ot + 1) * 128],
                        qT[:, ts(rt, THALF)],
                        start=(rt == 0),
                        stop=(rt == NR - 1),
                    )
                tmp = outp.tile([128, THALF], f32, tag="deq1")
                nc.vector.tensor_tensor(tmp[:], ops[:], lsb[:], ALU.mult)
                ob = outp.tile([128, THALF], f32, tag="deq2")
                nc.scalar.activation(
                    out=ob[:], in_=tmp[:], func=AF.Identity,
                    bias=bias_pp[:, ot : ot + 1], scale=ascale_pp[:, ot : ot + 1],
                )
                nc.sync.dma_start(
                    out=out_d[ts(ot, 128), ds(th * THALF, THALF)], in_=ob[:]
                )

    nc.compile()
    return nc


def _get_nc():
    global _compiled_nc
    if _compiled_nc is None:
        _compiled_nc = _build_nc()
    return _compiled_nc


def _make_in_maps(x, B_w, A_w, A_bias):
    x = np.asarray(x, dtype=np.float32).reshape(-1, IN)
    B_w = np.asarray(B_w, dtype=np.float32)
    A_w = np.ascontiguousarray(np.asarray(A_w, dtype=np.float32))
    A_bias = np.ascontiguousarray(np.asarray(A_bias, dtype=np.float32))
    bwt16 = np.ascontiguousarray(B_w.astype(np.float16).T)  # [IN, RANK]
    ident = np.eye(128, dtype=np.float32)
    ones_row = np.ones((1, 128), dtype=np.float32)
    in_maps = []
    for c in range(N_CORES):
        xt16 = np.ascontiguousarray(
            x[c * TOK : (c + 1) * TOK].astype(np.float16).T
        )  # [IN, TOK]
        in_maps.append(
            {
                "xT": xt16,
                "B_wT": bwt16,
                "A_w": A_w,
                "A_bias": A_bias,
                "ident": ident,
                "ones_row": ones_row,
            }
        )
    return in_maps


def _run(inputs, trace=False, trace_kwargs=None):
    from concourse.bass_utils import run_bass_kernel_spmd

    nc = _get_nc()
    in_maps = _make_in_maps(
        inputs["x"], inputs["B_w"], inputs["A_w"], inputs["A_bias"]
    )
    res = run_bass_kernel_spmd(
        nc, in_maps, core_ids=list(range(N_CORES)), trace=trace,
        **(trace_kwargs or {}),
    )
    parts = [res.results[c]["out"].T for c in range(N_CORES)]  # each [TOK, OUT]
    out = np.concatenate(parts, axis=0).reshape(B_SZ, SEQ, OUT)
    return np.ascontiguousarray(out.astype(np.float32)), res


def kernel(**inputs) -> np.ndarray:
    out, _ = _run(inputs, trace=False)
    return out
